# revision 1
# baseline (speedup 1.0000x reference)
"""Cross-attention Bass/Tile kernel for Trainium2, sharded over 8 NeuronCores.

Problem (fixed shapes): B=2, T=2048, C=1024, H=16 heads, D=64.
    q = x_q @ Wq + bq;  kv = x_kv @ Wkv + bkv;  k, v = split(kv)
    y = softmax(q k^T / sqrt(D)) v;  out = y @ Wo + bo

Sharding: 8 cores = 2 (batch) x 4 (head groups of 4 heads, 256 channels).
Each core computes its head-group's projections + attention + a partial
output projection (its 256 rows of Wo); the host sums the 4 partials per
batch.  The v-bias and output bias are folded in exactly on the host:
    y = att@(V + 1*bv) = att@V + 1*bv   (att rows sum to 1)
    => out += bv @ Wo + bo              (added once per batch on the host)

All matmuls fp32r (4-byte storage, ~1e-4 matmul precision, full PE rate
at free-dim >= 256).  PSUM budget (8 banks): 2 x [128,1024] "s" slots +
4 x [128,512] "y" slots, shared by every stage.

Emission schedule (PE executes in order, so emission order is the
schedule):
  Phase A: all K^T/V prep (PE transposes of x_kv^T, K/V projections),
           then Q-prep for tq=0.
  Phase B: per (tq, head-pair) pass over tk: S^T matmul (2 heads
           row-packed via tile_position) -> exp on ScalarE (scale=1/8
           in the activation affine) -> att@V matmuls lagging LAG steps
           (the S/exp prologue keeps ACT fed through pass boundaries).
           Only 2 y-banks accumulate per pass; the 2 spare y-banks run
           woven work units: Q-prep for tq+1 and the deferred output
           projection of tq-1.  Denominators ride along as row 64 of
           the y psum via a ones-column appended to V; normalization is
           reciprocal (fp32r) + K=1 broadcast matmul + DVE multiply.
"""

import numpy as np

B = 2
T = 2048
C = 1024
H = 16
D = 64
NCORES = 8
TPG = 4  # tensor-parallel group size (head groups)
HL = H // TPG  # heads per core = 4
CL = HL * D  # local channels = 256
P = 128

_CACHE = {}


def _build():
    import concourse.tile as tile
    from concourse import bacc, mybir
    from concourse.masks import make_identity

    f32 = mybir.dt.float32
    f32r = mybir.dt.float32r
    Exp = mybir.ActivationFunctionType.Exp

    nc = bacc.Bacc("TRN2", target_bir_lowering=False, debug=False)

    xq_d = nc.dram_tensor("xq", [T, C], f32, kind="ExternalInput")
    xkv_d = nc.dram_tensor("xkv", [T, C], f32, kind="ExternalInput")
    wq_d = nc.dram_tensor("wq", [C, CL], f32, kind="ExternalInput")
    wk_d = nc.dram_tensor("wk", [C, CL], f32, kind="ExternalInput")
    wv_d = nc.dram_tensor("wv", [C, CL], f32, kind="ExternalInput")
    wo_d = nc.dram_tensor("wo", [CL, C], f32, kind="ExternalInput")
    bq_d = nc.dram_tensor("bq", [CL], f32, kind="ExternalInput")
    bk_d = nc.dram_tensor("bk", [CL], f32, kind="ExternalInput")
    out_d = nc.dram_tensor("out", [T, C], f32, kind="ExternalOutput")

    KC = C // P  # 8 contraction chunks for the projections
    NT = T // P  # 16 token chunks of 128
    NQ = 4  # tq chunks of 512
    QW = T // NQ  # 512
    DC = CL // P  # 2 chunks of d_local
    LAG = 5

    with tile.TileContext(nc) as tc:
        with (
            tc.tile_pool(name="const", bufs=1) as const,
            tc.tile_pool(name="persist", bufs=1) as persist,
            tc.tile_pool(name="xnat", bufs=4) as xnat,
            tc.tile_pool(name="xt", bufs=1) as xtp,
            tc.tile_pool(name="ework", bufs=7) as ework,
            tc.tile_pool(name="norm2", bufs=1) as norm2,
            tc.tile_pool(name="outst", bufs=3) as outst,
        ):
            # ---- constants / weights (weights via SWDGE so HWDGE is free
            # for the x loads) ----
            ident = const.tile([P, P], f32)
            make_identity(nc, ident)
            identr = const.tile([P, P], f32r)
            nc.vector.tensor_copy(identr, ident)
            ones4_f32 = const.tile([P, HL, 1], f32)
            nc.vector.memset(ones4_f32, 1.0)
            onesb_f32 = const.tile([P, 64], f32)
            nc.vector.memset(onesb_f32, 1.0)
            onesb = const.tile([P, 64], f32r)
            nc.vector.tensor_copy(onesb, onesb_f32)

            wq_sb = const.tile([P, KC, CL], f32r)
            wk_sb = const.tile([P, KC, CL], f32r)
            wv_sb = const.tile([P, KC, CL], f32r)
            wo_sb = const.tile([P, DC, C], f32r)
            for w_sb, w_d in ((wk_sb, wk_d), (wv_sb, wv_d), (wq_sb, wq_d)):
                src = w_d.rearrange("(o p) d -> p o d", p=P).bitcast(f32r)
                for kc in range(KC):
                    nc.gpsimd.dma_start(w_sb[:, kc, :], src[:, kc, :])
            wo_src = wo_d.rearrange("(o p) n -> p o n", p=P).bitcast(f32r)
            for dc in range(DC):
                nc.gpsimd.dma_start(wo_sb[:, dc, :], wo_src[:, dc, :])
            bq_sb = const.tile([P, DC], f32)
            bk_sb = const.tile([P, DC], f32)
            nc.gpsimd.dma_start(bq_sb, bq_d.rearrange("(o p) -> p o", p=P))
            nc.gpsimd.dma_start(bk_sb, bk_d.rearrange("(o p) -> p o", p=P))

            # ---- persistent activations ----
            qt_sb = persist.tile([P, DC, T], f32r)  # Q^T  [d, t]
            kt_sb = persist.tile([P, DC, T], f32r)  # K^T  [d, t]
            v_sb = persist.tile([P, NT, HL, 66], f32r)  # V|1 [t, h, d+1]
            yt_sb = persist.tile([P, DC, T], f32r)  # y^T  [d, t] (normalized)

            # ---- kernel-wide PSUM: 2 x [128,1024] (s) + 4 x [128,512] (y)
            ps_s = tc.alloc_tile_pool(name="ps_s", bufs=2, space="PSUM")
            ps_y = tc.alloc_tile_pool(name="ps_y", bufs=4, space="PSUM")

            # ---------- emission helpers ----------
            def q_prep_units(tq):
                """Work units (thunks) producing xq^T and Q^T for `tq`."""
                xq_t = xtp.tile([P, KC, QW], f32r, tag="xqT", name="xq_t")
                units = []
                state = {}
                for ts_ in range(4):
                    tch = tq * 4 + ts_

                    def dma_u(ts_=ts_, tch=tch):
                        x_nat = xnat.tile([P, C], f32r, tag="xq_nat", name="x_nat")
                        state[ts_] = x_nat
                        for pc in range(2):
                            csl = slice(pc * 512, (pc + 1) * 512)
                            nc.sync.dma_start(
                                x_nat[:, csl], xq_d[tch * P : (tch + 1) * P, csl].bitcast(f32r)
                            )

                    units.append(dma_u)
                    for grp in range(2):

                        def tr_u(ts_=ts_, grp=grp):
                            x_nat = state[ts_]
                            tp = ps_y.tile([P, 4 * P], f32r, tag="y", name="tp")
                            for cc in range(4):
                                c = grp * 4 + cc
                                nc.tensor.transpose(
                                    tp[:, cc * P : (cc + 1) * P],
                                    x_nat[:, c * P : (c + 1) * P],
                                    identr,
                                )
                            nc.vector.tensor_copy(
                                xq_t[
                                    :, grp * 4 : (grp + 1) * 4, ts_ * P : (ts_ + 1) * P
                                ],
                                tp.rearrange("p (c t) -> p c t", c=4),
                            )

                        units.append(tr_u)
                for dc in range(DC):

                    def proj_u(dc=dc):
                        pp = ps_y.tile([P, QW], f32, tag="y", name="pp")
                        for c in range(KC):
                            nc.tensor.matmul(
                                pp,
                                wq_sb[:, c, dc * P : (dc + 1) * P],
                                xq_t[:, c, :],
                                start=(c == 0),
                                stop=(c == KC - 1),
                            )
                        nc.vector.tensor_scalar_add(
                            qt_sb[:, dc, tq * QW : (tq + 1) * QW],
                            pp,
                            bq_sb[:, dc : dc + 1],
                        )

                    units.append(proj_u)
                return units

            def po_units(tq, on_act=False):
                """Output-projection work units for `tq` (yt must be final)."""
                units = []
                for ts_ in range(4):
                    tch = tq * 4 + ts_
                    for co in range(2):

                        def u(tch=tch, co=co):
                            po = ps_y.tile([P, QW], f32, tag="y", name="po")
                            for dc in range(DC):
                                nc.tensor.matmul(
                                    po,
                                    yt_sb[:, dc, tch * P : (tch + 1) * P],
                                    wo_sb[:, dc, co * QW : (co + 1) * QW],
                                    start=(dc == 0),
                                    stop=(dc == DC - 1),
                                )
                            o_st = outst.tile([P, QW], f32, tag="o")
                            if on_act:
                                nc.scalar.copy(o_st, po)
                            else:
                                nc.vector.tensor_copy(o_st, po)
                            nc.sync.dma_start(
                                out_d[
                                    tch * P : (tch + 1) * P, co * QW : (co + 1) * QW
                                ],
                                o_st,
                            )

                        units.append(u)
                return units

            # ---- phase A: all K^T and V prep (V per token-chunk and K in
            # 256-halves, emitted as soon as their transposes exist so PE
            # fills the DMA waits) ----
            for tq in range(NQ):
                xkv_t = xtp.tile([P, KC, QW], f32r, tag="xkvT", name="xkv_t")
                for ts_ in range(4):
                    tch = tq * 4 + ts_
                    kv_nat = xnat.tile([P, C], f32r, tag="xkv_nat", name="kv_nat")
                    for pc in range(2):
                        csl = slice(pc * 512, (pc + 1) * 512)
                        nc.sync.dma_start(
                            kv_nat[:, csl],
                            xkv_d[tch * P : (tch + 1) * P, csl].bitcast(f32r),
                        )
                    tp = ps_s.tile([P, KC * P], f32r, tag="s", name="tp8")
                    for c in range(KC):
                        nc.tensor.transpose(
                            tp[:, c * P : (c + 1) * P],
                            kv_nat[:, c * P : (c + 1) * P],
                            identr,
                        )
                    nc.vector.tensor_copy(
                        xkv_t[:, :, ts_ * P : (ts_ + 1) * P],
                        tp.rearrange("p (c t) -> p c t", c=KC),
                    )
                    # V projection for this token chunk
                    pv = ps_y.tile([P, QW], f32, tag="y", name="pv")
                    for c in range(KC):
                        nc.tensor.matmul(
                            pv[:, :CL],
                            xkv_t[:, c, ts_ * P : (ts_ + 1) * P],
                            wv_sb[:, c, :],
                            start=(c == 0),
                            stop=(c == KC - 1),
                        )
                    nc.vector.tensor_copy(
                        v_sb[:, tch, :, 0:64],
                        pv[:, :CL].rearrange("p (h d) -> p h d", h=HL),
                    )
                    nc.vector.tensor_copy(v_sb[:, tch, :, 64:65], ones4_f32)
                    # K projection in 256-wide halves once 2 chunks exist
                    if ts_ in (1, 3):
                        half = ts_ // 2
                        hsl = slice(half * 256, (half + 1) * 256)
                        for dc in range(DC):
                            pp = ps_y.tile([P, QW], f32, tag="y", name="ppk")
                            for c in range(KC):
                                nc.tensor.matmul(
                                    pp[:, :256],
                                    wk_sb[:, c, dc * P : (dc + 1) * P],
                                    xkv_t[:, c, hsl],
                                    start=(c == 0),
                                    stop=(c == KC - 1),
                                )
                            nc.vector.tensor_scalar_add(
                                kt_sb[:, dc, tq * QW + half * 256 : tq * QW + (half + 1) * 256],
                                pp[:, :256],
                                bk_sb[:, dc : dc + 1],
                            )

            for u in q_prep_units(0):
                u()

            # ---- phase B: attention passes per (tq, head-pair) ----
            y_tiles = {}
            e_tiles = {}

            def emit_sexp(k, hc, tk):
                sp = ps_s.tile([P, 2 * QW], f32, tag="s", name="sp")
                for hh in range(2):
                    nc.tensor.matmul(
                        sp[:, hh * QW : (hh + 1) * QW],
                        kt_sb[hh * 64 : (hh + 1) * 64, hc, tk * P : (tk + 1) * P],
                        qt_sb[hh * 64 : (hh + 1) * 64, hc, k * QW : (k + 1) * QW],
                        start=True,
                        stop=True,
                        tile_position=(hh * 64, 0),
                    )
                e2 = ework.tile([P, 2 * QW], f32r, tag="e", name="e2")
                nc.scalar.activation(e2, sp, Exp, scale=0.125)
                e_tiles[(k, hc, tk)] = e2

            def emit_y(k, hc, tk):
                if (k, hc) not in y_tiles:
                    y_tiles[(k, hc)] = [
                        ps_y.tile([65, QW], f32, tag="y", name=f"y_ps{i}")
                        for i in range(2)
                    ]
                y_pair = y_tiles[(k, hc)]
                e2 = e_tiles.pop((k, hc, tk))
                for hh in range(2):
                    h = 2 * hc + hh
                    nc.tensor.matmul(
                        y_pair[hh],
                        v_sb[:, tk, h, :65],
                        e2[:, hh * QW : (hh + 1) * QW],
                        start=(tk == 0),
                        stop=(tk == NT - 1),
                    )

            def emit_norm(k, hc):
                y_pair = y_tiles.pop((k, hc))
                recr = norm2.tile([P, 2, QW], f32r, tag="recr")
                with nc.allow_low_precision(reason="fp32r reciprocal for bcast"):
                    for hh in range(2):
                        nc.vector.reciprocal(
                            recr[64:65, hh, :], y_pair[hh][64:65, :]
                        )
                rbp = ps_s.tile([P, 2 * QW], f32, tag="s", name="rbp")
                for hh in range(2):
                    nc.tensor.matmul(
                        rbp[0:64, hh * QW : (hh + 1) * QW],
                        onesb[64:65, :],
                        recr[64:65, hh, :],
                        start=True,
                        stop=True,
                        tile_position=(64, 0),
                        skip_group_check=True,
                    )
                rbs = norm2.tile([P, 2 * QW], f32, tag="rbs")
                nc.vector.tensor_copy(rbs[0:64, :], rbp[0:64, :])
                for hh in range(2):
                    rb_h = rbs[0:64, hh * QW : (hh + 1) * QW]
                    if hh == 0:
                        nc.vector.tensor_mul(
                            out=yt_sb[0:64, hc, k * QW : (k + 1) * QW],
                            in0=y_pair[hh][0:64, :],
                            in1=rb_h,
                        )
                    else:
                        yst = norm2.tile([64, QW], f32r, tag="yst")
                        nc.vector.tensor_mul(
                            out=yst, in0=y_pair[hh][0:64, :], in1=rb_h
                        )
                        nc.sync.dma_start(
                            yt_sb[64:128, hc, k * QW : (k + 1) * QW], yst
                        )

            passes = [(k, hc) for k in range(NQ) for hc in range(DC)]
            unit_q = []
            yq = []
            for pi, (k, hc) in enumerate(passes):
                if hc == 0:
                    # everything queued so far (prior Q-prep and outproj)
                    # must land before this tq's attention reads Q^T
                    while unit_q:
                        unit_q.pop(0)()
                    if k + 1 < NQ:
                        unit_q.extend(q_prep_units(k + 1))
                for tk in range(NT):
                    emit_sexp(k, hc, tk)
                    yq.append((k, hc, tk))
                    if len(yq) > LAG:
                        emit_y(*yq.pop(0))
                    if tk == 1 and pi >= 1:
                        pk, phc = passes[pi - 1]
                        while yq and yq[0][:2] == (pk, phc):
                            emit_y(*yq.pop(0))
                        emit_norm(pk, phc)
                        if hc == 0 and k >= 1:
                            unit_q.extend(po_units(k - 1))
                    if unit_q:
                        unit_q.pop(0)()
            while unit_q:
                unit_q.pop(0)()
            while yq:
                emit_y(*yq.pop(0))
            emit_norm(NQ - 1, DC - 1)
            for u in po_units(NQ - 1):
                u()

            ps_y.release()
            ps_s.release()

    nc.compile()
    return nc


def _get_nc():
    if "nc" not in _CACHE:
        _CACHE["nc"] = _build()
    return _CACHE["nc"]


def _shard_inputs(x_q, x_kv, Wq, bq, Wkv, bkv):
    in_maps = []
    for core in range(NCORES):
        b = core // TPG
        g = core % TPG
        cols = slice(g * CL, (g + 1) * CL)
        in_maps.append(
            {
                "xq": np.ascontiguousarray(x_q[b]),
                "xkv": np.ascontiguousarray(x_kv[b]),
                "wq": np.ascontiguousarray(Wq[:, cols]),
                "wk": np.ascontiguousarray(Wkv[:, :C][:, cols]),
                "wv": np.ascontiguousarray(Wkv[:, C:][:, cols]),
                "wo": None,  # filled by caller (needs Wo)
                "bq": np.ascontiguousarray(bq[cols]),
                "bk": np.ascontiguousarray(bkv[:C][cols]),
            }
        )
    return in_maps


def kernel(x_q, x_kv, Wq, bq, Wkv, bkv, Wo, bo):
    from concourse.bass_utils import run_bass_kernel_spmd

    x_q = np.asarray(x_q, dtype=np.float32)
    x_kv = np.asarray(x_kv, dtype=np.float32)
    Wq = np.asarray(Wq, dtype=np.float32)
    bq = np.asarray(bq, dtype=np.float32)
    Wkv = np.asarray(Wkv, dtype=np.float32)
    bkv = np.asarray(bkv, dtype=np.float32)
    Wo = np.asarray(Wo, dtype=np.float32)
    bo = np.asarray(bo, dtype=np.float32)

    nc = _get_nc()
    in_maps = _shard_inputs(x_q, x_kv, Wq, bq, Wkv, bkv)
    for core in range(NCORES):
        g = core % TPG
        in_maps[core]["wo"] = np.ascontiguousarray(Wo[g * CL : (g + 1) * CL, :])

    res = run_bass_kernel_spmd(nc, in_maps, core_ids=list(range(NCORES)))

    # host-side gather: sum tensor-parallel partials; add exact bias terms
    bias_full = bkv[C:] @ Wo + bo  # v-bias through Wo, plus output bias
    out = np.zeros((B, T, C), dtype=np.float32)
    for core in range(NCORES):
        out[core // TPG] += res.results[core]["out"]
    out += bias_full[None, None, :]
    return out



# revision 51
# speedup vs baseline: 1.2884x; 1.2884x over previous
"""Cross-attention Bass/Tile kernel for Trainium2, sharded over 8 NeuronCores.

Problem (fixed shapes): B=2, T=2048, C=1024, H=16 heads, D=64.
    q = x_q @ Wq + bq;  kv = x_kv @ Wkv + bkv;  k, v = split(kv)
    y = softmax(q k^T / sqrt(D)) v;  out = y @ Wo + bo
Sharding: 8 cores = 2 (batch) x 4 (head groups of 4 heads, 256 channels).
Each core computes its head-group's projections + attention + a partial
output projection; the host sums the 4 partials per batch and folds the
v-bias/output bias in exactly:
    y = att@(V + 1*bv) = att@V + 1*bv   (att rows sum to 1)
    => out += bv @ Wo + bo              (added once per batch on the host)

Precision: the host casts x and W to bf16.  All matmuls run bf16
(full PE rate at any free width); psum accumulation is fp32.
Measured end-to-end rel err ~1e-3 (tolerance 2e-2).

Key structure (cost-model-driven):
  - x_q^T / x_kv^T are produced directly by DMA-engine transposes
    (dma_start_transpose, 16x128 xbar tiles) from bf16 DRAM -- no PE
    transpose work and no DVE evacuation copies.
  - S^T = K^T.T Q^T per head-pair, two heads row-packed via
    tile_position (64-row contraction each), psum [128,1024] (2 banks),
    ping-ponged; exp on the ACT engine (scale=1/8 folded in) writes
    bf16 e2 tiles.  ACT is the structural bottleneck: 16.7M exps/core.
  - att@V uses the transposed mapping out[q, d]: lhsT = e2 block
    (stationary, [128k x 128q]), rhs = V chunk [128k x 64d]; 16
    accumulation steps of free width 64 -- half the PE cost of the
    [d, q] mapping.  Denominators via ones-column matmuls (N=1).
  - Normalization is fused into psum evacuation: DVE reciprocal of the
    den row + per-partition tensor_scalar_mul psum->SBUF (bf16).
  - y^T for the output projection comes from SBUF->SBUF DMA transposes.
  - PSUM: 2 x [128,1024] S slots (4 banks) + y accum (1 bank) + den
    (1 bank) + 2 woven work banks (K/Q/V projections + out-proj).
"""

import numpy as np

B = 2
T = 2048
C = 1024
H = 16
D = 64
NCORES = 8
TPG = 4  # tensor-parallel group size (head groups)
HL = H // TPG  # heads per core = 4
CL = HL * D  # local channels = 256
P = 128

_CACHE = {}


def _build():
    import concourse.tile as tile
    from concourse import bacc, mybir
    from concourse.masks import make_identity

    f32 = mybir.dt.float32
    bf16 = mybir.dt.bfloat16
    Exp = mybir.ActivationFunctionType.Exp

    nc = bacc.Bacc("TRN2", target_bir_lowering=False, debug=False)

    xq_d = nc.dram_tensor("xq", [T, C], bf16, kind="ExternalInput")
    xkv_d = nc.dram_tensor("xkv", [T, C], bf16, kind="ExternalInput")
    wq_d = nc.dram_tensor("wq", [C, CL], bf16, kind="ExternalInput")
    wk_d = nc.dram_tensor("wk", [C, CL], bf16, kind="ExternalInput")
    wv_d = nc.dram_tensor("wv", [C, CL], bf16, kind="ExternalInput")
    wo_d = nc.dram_tensor("wo", [CL, C], bf16, kind="ExternalInput")
    bq_d = nc.dram_tensor("bq", [CL], f32, kind="ExternalInput")
    bk_d = nc.dram_tensor("bk", [CL], f32, kind="ExternalInput")
    out_d = nc.dram_tensor("out", [T, C], bf16, kind="ExternalOutput")

    KC = C // P  # 8 contraction chunks for the projections
    NT = T // P  # 16 k-token chunks of 128
    NQ = 4  # tq chunks of 512
    QW = T // NQ  # 512
    DC = CL // P  # 2 head-pair chunks of d_local
    LAG = 5

    with tile.TileContext(nc) as tc:
        with (
            tc.tile_pool(name="const", bufs=1) as const,
            tc.tile_pool(name="persist", bufs=1) as persist,
            tc.tile_pool(name="ework", bufs=12) as ework,

            tc.tile_pool(name="ynorm", bufs=2) as ynorm,
            tc.tile_pool(name="outst", bufs=3) as outst,
        ):
            # ---- constants / weights ----
            ones1 = const.tile([P, 1], bf16)
            nc.vector.memset(ones1, 1.0)
            zwarm = const.tile([P, QW], bf16)
            nc.vector.memset(zwarm, 0.0)
            ident = const.tile([P, P], bf16)
            make_identity(nc, ident)

            wq_sb = const.tile([P, KC, CL], bf16)
            wk_sb = const.tile([P, KC, CL], bf16)
            wv_sb = const.tile([P, KC, CL], bf16)
            wo_sb = const.tile([P, DC, C], bf16)
            bq_sb = const.tile([P, DC], f32)
            bk_sb = const.tile([P, DC], f32)

            # ---- persistent activations ----
            # x^T and y^T live in per-chunk tiles: DMA writes into slices
            # of a single tile get serialized by WAW deps (each transfer
            # then pays ~2us of sem-prop + DGE latency); separate tiles
            # keep the DMA queue streaming back-to-back.
            xkv_tc = [persist.tile([P, KC, QW], bf16, name=f"xkv_t{i}") for i in range(NQ)]
            xq_tc = [persist.tile([P, KC, QW], bf16, name=f"xq_t{i}") for i in range(NQ)]
            kt_sb = persist.tile([P, DC, T], bf16)  # K^T    [d, t]
            qt_sb = persist.tile([P, DC, T], bf16)  # Q^T    [d, t]
            v_sb = persist.tile([P, NT, HL, D], bf16)  # V  [k, tch, h, d]
            # y^T per 128-token chunk: [d(128), hc, t(128)]
            yt_t = [persist.tile([P, DC, P], bf16, name=f"yt{i}") for i in range(NT)]

            # DMA scheduling: all transfers serialize on the DMA engines.
            # The framework inserts a ring-drain semaphore wait whenever
            # the DMA TYPE changes on a ring (xbar-transpose mode vs
            # plain copy), costing ~2.2us; type-pure rings stream
            # back-to-back.  So: SP ring = ONLY xbar transposes (all 8
            # x^T chunks); SWDGE (gpsimd) = all weight/bias copies and
            # mid-stream output writes; ACT ring = tail output writes
            # only (a queued ACT-ring DMA wait would stall the ACT
            # sequencer and with it every exp).
            nc.gpsimd.dma_start(bq_sb, bq_d.rearrange("(o p) -> p o", p=P))
            nc.gpsimd.dma_start(bk_sb, bk_d.rearrange("(o p) -> p o", p=P))
            nc.gpsimd.dma_start(wk_sb, wk_d.rearrange("(o p) d -> p o d", p=P))
            nc.sync.dma_start_transpose(xkv_tc[0], xkv_d[0:QW, :])
            nc.sync.dma_start_transpose(xq_tc[0], xq_d[0:QW, :])
            nc.gpsimd.dma_start(wq_sb, wq_d.rearrange("(o p) d -> p o d", p=P))
            nc.gpsimd.dma_start(wv_sb, wv_d.rearrange("(o p) d -> p o d", p=P))
            for tch in range(1, NQ):
                nc.sync.dma_start_transpose(
                    xkv_tc[tch], xkv_d[tch * QW : (tch + 1) * QW, :]
                )
            nc.gpsimd.dma_start(wo_sb, wo_d.rearrange("(o p) n -> p o n", p=P))
            nc.sync.dma_start_transpose(xq_tc[1], xq_d[QW : 2 * QW, :])
            nc.sync.dma_start_transpose(xq_tc[2], xq_d[2 * QW : 3 * QW, :])
            nc.sync.dma_start_transpose(xq_tc[3], xq_d[3 * QW : 4 * QW, :])

            # ---- kernel-wide PSUM ----
            ps_s = tc.alloc_tile_pool(name="ps_s", bufs=2, space="PSUM")
            ps_y = tc.alloc_tile_pool(name="ps_y", bufs=1, space="PSUM")
            ps_d = tc.alloc_tile_pool(name="ps_d", bufs=1, space="PSUM")
            ps_w = tc.alloc_tile_pool(name="ps_w", bufs=2, space="PSUM")

            # den bank is shared by all 8 passes (disjoint 8-column
            # slots): zero it once and accumulate with start=False
            # throughout.  A start=True matmul wipes its whole psum
            # bank, not just the addressed region, so interleaved
            # accumulation groups sharing a bank must not re-start.
            den_ps = ps_d.tile([P, 8 * 8], f32, tag="den")  # [q, pass*8+slot]
            nc.vector.memset(den_ps, 0.0)

            # ---------- work units (thunks) ----------
            def k_unit(dc, tch):
                """K^T chunk: out [128d, 512t] for head-pair dc."""

                def u():
                    pp = ps_w.tile([P, QW], f32, tag="w", name="ppk")
                    for c in range(KC):
                        nc.tensor.matmul(
                            pp,
                            wk_sb[:, c, dc * P : (dc + 1) * P],
                            xkv_tc[tch][:, c, :],
                            start=(c == 0),
                            stop=(c == KC - 1),
                        )
                    nc.vector.tensor_scalar_add(
                        kt_sb[:, dc, tch * QW : (tch + 1) * QW],
                        pp,
                        bk_sb[:, dc : dc + 1],
                    )

                return u

            def q_unit(dc, tq):
                def u():
                    pp = ps_w.tile([P, QW], f32, tag="w", name="ppq")
                    for c in range(KC):
                        nc.tensor.matmul(
                            pp,
                            wq_sb[:, c, dc * P : (dc + 1) * P],
                            xq_tc[tq][:, c, :],
                            start=(c == 0),
                            stop=(c == KC - 1),
                        )
                    nc.vector.tensor_scalar_add(
                        qt_sb[:, dc, tq * QW : (tq + 1) * QW],
                        pp,
                        bq_sb[:, dc : dc + 1],
                    )

                return u

            def v_unit(tk):
                """V chunk tk: out [128t, 256d] natural layout."""

                def u():
                    pv = ps_w.tile([P, QW], f32, tag="w", name="ppv")
                    for c in range(KC):
                        nc.tensor.matmul(
                            pv[:, :CL],
                            xkv_tc[tk // 4][:, c, (tk % 4) * P : (tk % 4 + 1) * P],
                            wv_sb[:, c, :],
                            start=(c == 0),
                            stop=(c == KC - 1),
                        )
                    nc.vector.tensor_copy(
                        v_sb[:, tk, :, :],
                        pv[:, :CL].rearrange("p (h d) -> p h d", h=HL),
                    )

                return u

            def po_unit(tch, dma_eng=None, on_act=False):
                """Output projection for 128 tokens: out[tch*128, :] whole
                row block, one batched DMA.  dma_eng picks the DMA ring
                (default SWDGE so the HWDGE rings stay clear; the final
                chunks use SP/ACT for lower tail latency).  on_act: evac
                the psum on the ACT engine (idle after the last exp)."""

                def u():
                    o_st = outst.tile([P, C], bf16, tag="o")
                    for co in range(2):
                        po = ps_w.tile([P, QW], f32, tag="w", name="po")
                        for dc in range(DC):
                            nc.tensor.matmul(
                                po,
                                yt_t[tch][:, dc, :],
                                wo_sb[:, dc, co * QW : (co + 1) * QW],
                                start=(dc == 0),
                                stop=(dc == DC - 1),
                            )
                        if on_act:
                            nc.scalar.copy(o_st[:, co * QW : (co + 1) * QW], po)
                        else:
                            nc.vector.tensor_copy(
                                o_st[:, co * QW : (co + 1) * QW], po
                            )
                    (dma_eng or nc.gpsimd).dma_start(
                        out_d[tch * P : (tch + 1) * P, :], o_st
                    )

                return u

            # ---------- attention pass machinery ----------
            y_tiles = {}
            e_tiles = {}

            def emit_sexp(k, hc, tk):
                sp = ps_s.tile([P, 2 * QW], f32, tag="s", name="sp")
                for hh in range(2):
                    nc.tensor.matmul(
                        sp[:, hh * QW : (hh + 1) * QW],
                        kt_sb[hh * 64 : (hh + 1) * 64, hc, tk * P : (tk + 1) * P],
                        qt_sb[hh * 64 : (hh + 1) * 64, hc, k * QW : (k + 1) * QW],
                        start=True,
                        stop=True,
                        tile_position=(hh * 64, 0),
                    )
                e2 = ework.tile([P, 2 * QW], bf16, tag="e", name="e2")
                nc.scalar.activation(e2, sp, Exp, scale=0.125)
                e_tiles[(k, hc, tk)] = e2

            def emit_y(k, hc, tk):
                fresh = (k, hc) not in y_tiles
                if fresh:
                    y_tiles[(k, hc)] = ps_y.tile(
                        [P, 8, D], f32, tag="y", name="y_ps"
                    )
                y_ps = y_tiles[(k, hc)]
                e2 = e_tiles.pop((k, hc, tk))
                ps8 = (k * DC + hc) * 8
                for hh in range(2):
                    h = 2 * hc + hh
                    for qb in range(4):
                        eblk = e2[:, hh * QW + qb * P : hh * QW + (qb + 1) * P]
                        # single start per pass: it zeroes the whole y
                        # bank (all 8 accumulation slots at once)
                        nc.tensor.matmul(
                            y_ps[:, hh * 4 + qb, :],
                            eblk,
                            v_sb[:, tk, h, :],
                            start=(fresh and hh == 0 and qb == 0),
                            stop=(tk == NT - 1),
                            skip_group_check=True,
                        )
                        nc.tensor.matmul(
                            den_ps[:, ps8 + hh * 4 + qb : ps8 + hh * 4 + qb + 1],
                            eblk,
                            ones1,
                            start=False,
                            stop=(tk == NT - 1),
                            skip_group_check=True,
                        )

            def emit_norm(k, hc, on_pe=True):
                """Evacuate + normalize pass (k, hc): y^T chunk into yt_t.

                on_pe: transpose via the PE (cheap: 512 rows bf16)
                instead of the DMA xbar, avoiding the per-DMA ring
                latency (~2.2us) on the path to the output projection.
                """
                y_ps = y_tiles.pop((k, hc))
                ps8 = (k * DC + hc) * 8
                rec = ynorm.tile([P, 8], f32, tag="rec")
                nc.vector.reciprocal(rec, den_ps[:, ps8 : ps8 + 8])
                yn = ynorm.tile([P, 4, 2 * D], bf16, tag="yn")
                for hh in range(2):
                    for qb in range(4):
                        nc.vector.tensor_scalar_mul(
                            yn[:, qb, hh * D : (hh + 1) * D],
                            y_ps[:, hh * 4 + qb, :],
                            rec[:, hh * 4 + qb : hh * 4 + qb + 1],
                        )
                if on_pe:
                    tp = ps_w.tile([P, QW], bf16, tag="w", name="ytp")
                    for qb in range(4):
                        nc.tensor.transpose(
                            tp[:, qb * P : (qb + 1) * P], yn[:, qb, :], ident
                        )
                    for qb in range(4):
                        nc.vector.tensor_copy(
                            yt_t[k * 4 + qb][:, hc, :],
                            tp[:, qb * P : (qb + 1) * P],
                        )
                else:
                    for qb in range(4):
                        nc.sync.dma_start_transpose(
                            yt_t[k * 4 + qb][:, hc, :], yn[:, qb, :]
                        )

            # ---------- emission schedule ----------
            # PE warm-up: the cost model's p-state ramp needs ~3us of
            # continuous PE execution for full clock; idle resets it.
            # Stream cheap matmuls on a zero tile until the first x
            # chunk lands so K/Q projections run at full speed.
            warm = ps_w.tile([P, QW], f32, tag="w", name="warm")
            for i in range(12):
                nc.tensor.matmul(
                    warm[0:1, :], zwarm[:, 0:1], zwarm, start=True, stop=True
                )

            # Startup: K^T chunk 0, PE-transpose of x_q(0), Q^T(tq0) ->
            # first exp asap.  The rest (K c1-c3, all V, later Q/PO)
            # weaves through pass slots.  Correctness: consumers
            # force-drain the unit queue up to their producer (the Tile
            # dep-tracker follows emission order, so a producer must be
            # emitted before its reader); the queue is constructed in
            # dependency order.
            k_unit(0, 0)()
            k_unit(1, 0)()
            q_unit(0, 0)()
            q_unit(1, 0)()

            unit_q = []
            unit_q.append((("k", 1), k_unit(0, 1)))
            unit_q.append((("k", 1), k_unit(1, 1)))
            for tk in range(2):
                unit_q.append((("v", tk), v_unit(tk)))
            unit_q.append((("k", 2), k_unit(0, 2)))
            unit_q.append((("k", 2), k_unit(1, 2)))
            for tk in range(2, 4):
                unit_q.append((("v", tk), v_unit(tk)))
            unit_q.append((("k", 3), k_unit(0, 3)))
            unit_q.append((("k", 3), k_unit(1, 3)))
            for tk in range(4, 16):
                unit_q.append((("v", tk), v_unit(tk)))

            def pop_unit():
                key, u = unit_q.pop(0)
                u()

            def ensure(key):
                # units sharing a key (both dc of a K/Q chunk) drain to
                # the LAST matching entry
                while any(k_ == key for k_, _ in unit_q):
                    pop_unit()

            def pop_y(lag):
                if len(yq) > lag:
                    kk, hh_, tt = yq.pop(0)
                    ensure(("v", tt))
                    emit_y(kk, hh_, tt)
                    return True
                return False

            passes = [(k, hc) for k in range(NQ) for hc in range(DC)]
            npass = len(passes)
            yq = []
            for pi, (k, hc) in enumerate(passes):
                last = pi == npass - 1
                lag = 8 if pi == 0 else (2 if last else LAG)
                ensure(("q", k))
                normed = pi == 0
                po_queued = pi == 0
                for tk in range(NT):
                    ensure(("k", tk // 4))
                    emit_sexp(k, hc, tk)
                    yq.append((k, hc, tk))
                    pop_y(lag)
                    if not normed and yq[0][:2] != passes[pi - 1]:
                        # previous pass fully drained by the natural pops
                        emit_norm(*passes[pi - 1])
                        normed = True
                    elif normed and not po_queued:
                        # norm done last slot; queue the output projection
                        # (hc==1 passes finalize tq k-1)
                        po_queued = True
                        if hc == 1 and k >= 1:
                            for tch in range(4):
                                unit_q.append(
                                    (("po", k - 1), po_unit((k - 1) * 4 + tch))
                                )
                    if tk == 2 and hc == 1 and k + 1 < NQ:
                        unit_q.append((("q", k + 1), q_unit(0, k + 1)))
                        unit_q.append((("q", k + 1), q_unit(1, k + 1)))
                    if len(yq) > lag + 2:
                        pop_y(lag)
                    if unit_q and (pi <= 1 or tk % 2 == 1) and (pi == 0 or tk > 2):
                        pop_unit()
                    if unit_q and len(unit_q) > 8 and tk % 2 == 0:
                        pop_unit()
                    if last and tk >= 12:
                        while unit_q:
                            pop_unit()
                        pop_y(2)
            while yq:
                kk, hh_, tt = yq.pop(0)
                ensure(("v", tt))
                emit_y(kk, hh_, tt)
            emit_norm(NQ - 1, DC - 1, on_pe=True)
            while unit_q:
                pop_unit()
            for tch in range(4):
                eng = nc.sync if tch % 2 == 0 else nc.scalar
                po_unit((NQ - 1) * 4 + tch, dma_eng=eng, on_act=True)()

            ps_w.release()
            ps_d.release()
            ps_y.release()
            ps_s.release()

    nc.compile()
    return nc


def _get_nc():
    if "nc" not in _CACHE:
        _CACHE["nc"] = _build()
    return _CACHE["nc"]


def _bf16(a):
    import ml_dtypes

    return np.ascontiguousarray(np.asarray(a, dtype=np.float32)).astype(
        ml_dtypes.bfloat16
    )


def _shard_inputs(x_q, x_kv, Wq, bq, Wkv, bkv):
    x_q = np.asarray(x_q, dtype=np.float32)
    x_kv = np.asarray(x_kv, dtype=np.float32)
    Wq = np.asarray(Wq, dtype=np.float32)
    bq = np.asarray(bq, dtype=np.float32)
    Wkv = np.asarray(Wkv, dtype=np.float32)
    bkv = np.asarray(bkv, dtype=np.float32)
    in_maps = []
    for core in range(NCORES):
        b = core // TPG
        g = core % TPG
        cols = slice(g * CL, (g + 1) * CL)
        in_maps.append(
            {
                "xq": _bf16(x_q[b]),
                "xkv": _bf16(x_kv[b]),
                "wq": _bf16(Wq[:, cols]),
                "wk": _bf16(Wkv[:, :C][:, cols]),
                "wv": _bf16(Wkv[:, C:][:, cols]),
                "wo": None,  # filled by caller (needs Wo)
                "bq": np.ascontiguousarray(bq[cols]),
                "bk": np.ascontiguousarray(bkv[:C][cols]),
            }
        )
    return in_maps


def kernel(x_q, x_kv, Wq, bq, Wkv, bkv, Wo, bo):
    from concourse.bass_utils import run_bass_kernel_spmd

    Wo = np.asarray(Wo, dtype=np.float32)
    bo = np.asarray(bo, dtype=np.float32)
    bkv = np.asarray(bkv, dtype=np.float32)

    nc = _get_nc()
    in_maps = _shard_inputs(x_q, x_kv, Wq, bq, Wkv, bkv)
    for core in range(NCORES):
        g = core % TPG
        in_maps[core]["wo"] = _bf16(Wo[g * CL : (g + 1) * CL, :])

    res = run_bass_kernel_spmd(nc, in_maps, core_ids=list(range(NCORES)))

    # host-side gather: sum tensor-parallel partials; add exact bias terms
    bias_full = bkv[C:] @ Wo + bo  # v-bias through Wo, plus output bias
    out = np.zeros((B, T, C), dtype=np.float32)
    for core in range(NCORES):
        out[core // TPG] += np.asarray(res.results[core]["out"]).astype(np.float32)
    out += bias_full[None, None, :]
    return out


# revision 58
# speedup vs baseline: 1.2893x; 1.0007x over previous
"""Cross-attention Bass/Tile kernel for Trainium2, sharded over 8 NeuronCores.

Problem (fixed shapes): B=2, T=2048, C=1024, H=16 heads, D=64.
    q = x_q @ Wq + bq;  kv = x_kv @ Wkv + bkv;  k, v = split(kv)
    y = softmax(q k^T / sqrt(D)) v;  out = y @ Wo + bo
Sharding: 8 cores = 2 (batch) x 4 (head groups of 4 heads, 256 channels).
Each core computes its head-group's projections + attention + a partial
output projection; the host sums the 4 partials per batch and folds the
v-bias/output bias in exactly:
    y = att@(V + 1*bv) = att@V + 1*bv   (att rows sum to 1)
    => out += bv @ Wo + bo              (added once per batch on the host)

Precision: the host casts x and W to bf16.  All matmuls run bf16
(full PE rate at any free width); psum accumulation is fp32.
Measured end-to-end rel err ~5e-3 (tolerance 2e-2).

Key structure (cost-model-driven):
  - x_q^T / x_kv^T are produced directly by DMA-engine transposes
    (dma_start_transpose, 16x128 xbar tiles) from bf16 DRAM -- no PE
    transpose work and no DVE evacuation copies.
  - S^T = K^T.T Q^T per head-pair, two heads row-packed via
    tile_position (64-row contraction each), psum [128,1024] (2 banks),
    ping-ponged; exp on the ACT engine (scale=1/8 folded in) writes
    bf16 e2 tiles.  ACT is the structural bottleneck: 16.7M exps/core.
  - att@V uses the transposed mapping out[q, d]: lhsT = e2 block
    (stationary, [128k x 128q]), rhs = V chunk [128k x 64d]; 16
    accumulation steps of free width 64 -- half the PE cost of the
    [d, q] mapping.  Denominators via ones-column matmuls (N=1).
  - Normalization is fused into psum evacuation: DVE reciprocal of the
    den row + per-partition tensor_scalar_mul psum->SBUF (bf16).
  - y^T for the output projection comes from PE transposes (bf16,
    512 rows/pass) -- cheaper than the per-DMA ring latency.
  - PSUM: 2 x [128,1024] S slots (4 banks) + y accum (1 bank) + den
    (1 bank) + 2 woven work banks (K/Q/V projections + out-proj).
"""

import numpy as np

B = 2
T = 2048
C = 1024
H = 16
D = 64
NCORES = 8
TPG = 4  # tensor-parallel group size (head groups)
HL = H // TPG  # heads per core = 4
CL = HL * D  # local channels = 256
P = 128

_CACHE = {}


def _build():
    import concourse.tile as tile
    from concourse import bacc, mybir
    from concourse.masks import make_identity

    f32 = mybir.dt.float32
    bf16 = mybir.dt.bfloat16
    Exp = mybir.ActivationFunctionType.Exp

    nc = bacc.Bacc("TRN2", target_bir_lowering=False, debug=False)

    xq_d = nc.dram_tensor("xq", [T, C], bf16, kind="ExternalInput")
    xkv_d = nc.dram_tensor("xkv", [T, C], bf16, kind="ExternalInput")
    wq_d = nc.dram_tensor("wq", [C, CL], bf16, kind="ExternalInput")
    wk_d = nc.dram_tensor("wk", [C, CL], bf16, kind="ExternalInput")
    wv_d = nc.dram_tensor("wv", [C, CL], bf16, kind="ExternalInput")
    wo_d = nc.dram_tensor("wo", [CL, C], bf16, kind="ExternalInput")
    bq_d = nc.dram_tensor("bq", [CL], f32, kind="ExternalInput")
    bk_d = nc.dram_tensor("bk", [CL], f32, kind="ExternalInput")
    out_d = nc.dram_tensor("out", [T, C], bf16, kind="ExternalOutput")

    KC = C // P  # 8 contraction chunks for the projections
    NT = T // P  # 16 k-token chunks of 128
    NQ = 4  # tq chunks of 512
    QW = T // NQ  # 512
    DC = CL // P  # 2 head-pair chunks of d_local
    LAG = 5

    with tile.TileContext(nc) as tc:
        with (
            tc.tile_pool(name="const", bufs=1) as const,
            tc.tile_pool(name="persist", bufs=1) as persist,
            tc.tile_pool(name="ework", bufs=12) as ework,

            tc.tile_pool(name="ynorm", bufs=2) as ynorm,
            tc.tile_pool(name="outst", bufs=3) as outst,
        ):
            # ---- constants / weights ----
            ones1 = const.tile([P, 1], bf16)
            nc.vector.memset(ones1, 1.0)
            zwarm = const.tile([P, QW], bf16)
            nc.vector.memset(zwarm, 0.0)
            ident = const.tile([P, P], bf16)
            make_identity(nc, ident)

            wq_sb = const.tile([P, KC, CL], bf16)
            wk_sb = const.tile([P, KC, CL], bf16)
            wv_sb = const.tile([P, KC, CL], bf16)
            wo_sb = const.tile([P, DC, C], bf16)
            bq_sb = const.tile([P, DC], f32)
            bk_sb = const.tile([P, DC], f32)

            # ---- persistent activations ----
            # x^T and y^T live in per-chunk tiles: DMA writes into slices
            # of a single tile get serialized by WAW deps (each transfer
            # then pays ~2us of sem-prop + DGE latency); separate tiles
            # keep the DMA queue streaming back-to-back.
            xkv_tc = [persist.tile([P, KC, QW], bf16, name=f"xkv_t{i}") for i in range(NQ)]
            xq_tc = [persist.tile([P, KC, QW], bf16, name=f"xq_t{i}") for i in range(NQ)]
            kt_sb = persist.tile([P, DC, T], bf16)  # K^T    [d, t]
            qt_sb = persist.tile([P, DC, T], bf16)  # Q^T    [d, t]
            v_sb = persist.tile([P, NT, HL, D], bf16)  # V  [k, tch, h, d]
            # y^T per 128-token chunk: [d(128), hc, t(128)]
            yt_t = [persist.tile([P, DC, P], bf16, name=f"yt{i}") for i in range(NT)]

            # DMA scheduling: all transfers serialize on the DMA engines.
            # The framework inserts a ring-drain semaphore wait whenever
            # the DMA TYPE changes on a ring (xbar-transpose mode vs
            # plain copy), costing ~2.2us; type-pure rings stream
            # back-to-back.  So: SP ring = ONLY xbar transposes (all 8
            # x^T chunks); SWDGE (gpsimd) = all weight/bias copies and
            # mid-stream output writes; ACT ring = tail output writes
            # only (a queued ACT-ring DMA wait would stall the ACT
            # sequencer and with it every exp).
            # biases ride the otherwise-empty ACT ring: first-on-ring,
            # no chain waits, so they land in ~2us and never gate the
            # K/Q evacuations (and never stall the ACT sequencer)
            nc.scalar.dma_start(bq_sb, bq_d.rearrange("(o p) -> p o", p=P))
            nc.scalar.dma_start(bk_sb, bk_d.rearrange("(o p) -> p o", p=P))
            nc.gpsimd.dma_start(wk_sb, wk_d.rearrange("(o p) d -> p o d", p=P))
            nc.sync.dma_start_transpose(xkv_tc[0], xkv_d[0:QW, :])
            nc.sync.dma_start_transpose(xq_tc[0], xq_d[0:QW, :])
            nc.gpsimd.dma_start(wq_sb, wq_d.rearrange("(o p) d -> p o d", p=P))
            nc.gpsimd.dma_start(wv_sb, wv_d.rearrange("(o p) d -> p o d", p=P))
            for tch in range(1, NQ):
                nc.sync.dma_start_transpose(
                    xkv_tc[tch], xkv_d[tch * QW : (tch + 1) * QW, :]
                )
            nc.gpsimd.dma_start(wo_sb, wo_d.rearrange("(o p) n -> p o n", p=P))
            nc.sync.dma_start_transpose(xq_tc[1], xq_d[QW : 2 * QW, :])
            nc.sync.dma_start_transpose(xq_tc[2], xq_d[2 * QW : 3 * QW, :])
            nc.sync.dma_start_transpose(xq_tc[3], xq_d[3 * QW : 4 * QW, :])

            # ---- kernel-wide PSUM ----
            ps_s = tc.alloc_tile_pool(name="ps_s", bufs=2, space="PSUM")
            ps_y = tc.alloc_tile_pool(name="ps_y", bufs=1, space="PSUM")
            ps_d = tc.alloc_tile_pool(name="ps_d", bufs=1, space="PSUM")
            ps_w = tc.alloc_tile_pool(name="ps_w", bufs=2, space="PSUM")

            # den bank is shared by all 8 passes (disjoint 8-column
            # slots): zero it once and accumulate with start=False
            # throughout.  A start=True matmul wipes its whole psum
            # bank, not just the addressed region, so interleaved
            # accumulation groups sharing a bank must not re-start.
            den_ps = ps_d.tile([P, 8 * 8], f32, tag="den")  # [q, pass*8+slot]
            nc.vector.memset(den_ps, 0.0)

            # ---------- work units (thunks) ----------
            def k_unit(dc, tch):
                """K^T chunk: out [128d, 512t] for head-pair dc."""

                def u():
                    pp = ps_w.tile([P, QW], f32, tag="w", name="ppk")
                    for c in range(KC):
                        nc.tensor.matmul(
                            pp,
                            wk_sb[:, c, dc * P : (dc + 1) * P],
                            xkv_tc[tch][:, c, :],
                            start=(c == 0),
                            stop=(c == KC - 1),
                        )
                    nc.vector.tensor_scalar_add(
                        kt_sb[:, dc, tch * QW : (tch + 1) * QW],
                        pp,
                        bk_sb[:, dc : dc + 1],
                    )

                return u

            def q_unit(dc, tq):
                def u():
                    pp = ps_w.tile([P, QW], f32, tag="w", name="ppq")
                    for c in range(KC):
                        nc.tensor.matmul(
                            pp,
                            wq_sb[:, c, dc * P : (dc + 1) * P],
                            xq_tc[tq][:, c, :],
                            start=(c == 0),
                            stop=(c == KC - 1),
                        )
                    nc.vector.tensor_scalar_add(
                        qt_sb[:, dc, tq * QW : (tq + 1) * QW],
                        pp,
                        bq_sb[:, dc : dc + 1],
                    )

                return u

            def v_unit(tk):
                """V chunk tk: out [128t, 256d] natural layout."""

                def u():
                    pv = ps_w.tile([P, QW], f32, tag="w", name="ppv")
                    for c in range(KC):
                        nc.tensor.matmul(
                            pv[:, :CL],
                            xkv_tc[tk // 4][:, c, (tk % 4) * P : (tk % 4 + 1) * P],
                            wv_sb[:, c, :],
                            start=(c == 0),
                            stop=(c == KC - 1),
                        )
                    nc.vector.tensor_copy(
                        v_sb[:, tk, :, :],
                        pv[:, :CL].rearrange("p (h d) -> p h d", h=HL),
                    )

                return u

            def po_unit(tch, dma_eng=None, on_act=False):
                """Output projection for 128 tokens: out[tch*128, :] whole
                row block, one batched DMA.  dma_eng picks the DMA ring
                (default SWDGE so the HWDGE rings stay clear; the final
                chunks use SP/ACT for lower tail latency).  on_act: evac
                the psum on the ACT engine (idle after the last exp)."""

                def u():
                    o_st = outst.tile([P, C], bf16, tag="o")
                    for co in range(2):
                        po = ps_w.tile([P, QW], f32, tag="w", name="po")
                        for dc in range(DC):
                            nc.tensor.matmul(
                                po,
                                yt_t[tch][:, dc, :],
                                wo_sb[:, dc, co * QW : (co + 1) * QW],
                                start=(dc == 0),
                                stop=(dc == DC - 1),
                            )
                        if on_act:
                            nc.scalar.copy(o_st[:, co * QW : (co + 1) * QW], po)
                        else:
                            nc.vector.tensor_copy(
                                o_st[:, co * QW : (co + 1) * QW], po
                            )
                    (dma_eng or nc.gpsimd).dma_start(
                        out_d[tch * P : (tch + 1) * P, :], o_st
                    )

                return u

            # ---------- attention pass machinery ----------
            y_tiles = {}
            e_tiles = {}

            def emit_sexp(k, hc, tk):
                sp = ps_s.tile([P, 2 * QW], f32, tag="s", name="sp")
                for hh in range(2):
                    nc.tensor.matmul(
                        sp[:, hh * QW : (hh + 1) * QW],
                        kt_sb[hh * 64 : (hh + 1) * 64, hc, tk * P : (tk + 1) * P],
                        qt_sb[hh * 64 : (hh + 1) * 64, hc, k * QW : (k + 1) * QW],
                        start=True,
                        stop=True,
                        tile_position=(hh * 64, 0),
                    )
                e2 = ework.tile([P, 2 * QW], bf16, tag="e", name="e2")
                nc.scalar.activation(e2, sp, Exp, scale=0.125)
                e_tiles[(k, hc, tk)] = e2

            def emit_y(k, hc, tk):
                fresh = (k, hc) not in y_tiles
                if fresh:
                    y_tiles[(k, hc)] = ps_y.tile(
                        [P, 8, D], f32, tag="y", name="y_ps"
                    )
                y_ps = y_tiles[(k, hc)]
                e2 = e_tiles.pop((k, hc, tk))
                ps8 = (k * DC + hc) * 8
                for hh in range(2):
                    h = 2 * hc + hh
                    for qb in range(4):
                        eblk = e2[:, hh * QW + qb * P : hh * QW + (qb + 1) * P]
                        # single start per pass: it zeroes the whole y
                        # bank (all 8 accumulation slots at once)
                        nc.tensor.matmul(
                            y_ps[:, hh * 4 + qb, :],
                            eblk,
                            v_sb[:, tk, h, :],
                            start=(fresh and hh == 0 and qb == 0),
                            stop=(tk == NT - 1),
                            skip_group_check=True,
                        )
                        nc.tensor.matmul(
                            den_ps[:, ps8 + hh * 4 + qb : ps8 + hh * 4 + qb + 1],
                            eblk,
                            ones1,
                            start=False,
                            stop=(tk == NT - 1),
                            skip_group_check=True,
                        )

            def emit_norm(k, hc, on_pe=True):
                """Evacuate + normalize pass (k, hc): y^T chunk into yt_t.

                on_pe: transpose via the PE (cheap: 512 rows bf16)
                instead of the DMA xbar, avoiding the per-DMA ring
                latency (~2.2us) on the path to the output projection.
                """
                y_ps = y_tiles.pop((k, hc))
                ps8 = (k * DC + hc) * 8
                rec = ynorm.tile([P, 8], f32, tag="rec")
                nc.vector.reciprocal(rec, den_ps[:, ps8 : ps8 + 8])
                yn = ynorm.tile([P, 4, 2 * D], bf16, tag="yn")
                for hh in range(2):
                    for qb in range(4):
                        nc.vector.tensor_scalar_mul(
                            yn[:, qb, hh * D : (hh + 1) * D],
                            y_ps[:, hh * 4 + qb, :],
                            rec[:, hh * 4 + qb : hh * 4 + qb + 1],
                        )
                if on_pe:
                    tp = ps_w.tile([P, QW], bf16, tag="w", name="ytp")
                    for qb in range(4):
                        nc.tensor.transpose(
                            tp[:, qb * P : (qb + 1) * P], yn[:, qb, :], ident
                        )
                    for qb in range(4):
                        nc.vector.tensor_copy(
                            yt_t[k * 4 + qb][:, hc, :],
                            tp[:, qb * P : (qb + 1) * P],
                        )
                else:
                    for qb in range(4):
                        nc.sync.dma_start_transpose(
                            yt_t[k * 4 + qb][:, hc, :], yn[:, qb, :]
                        )

            # ---------- emission schedule ----------
            # PE warm-up: the cost model's p-state ramp needs ~3us of
            # continuous PE execution for full clock; idle resets it.
            # Stream cheap matmuls on a zero tile until the first x
            # chunk lands so K/Q projections run at full speed.
            warm = ps_w.tile([P, QW], f32, tag="w", name="warm")
            for i in range(12):
                nc.tensor.matmul(
                    warm[0:1, :], zwarm[:, 0:1], zwarm, start=True, stop=True
                )

            # Startup: K^T chunk 0, PE-transpose of x_q(0), Q^T(tq0) ->
            # first exp asap.  The rest (K c1-c3, all V, later Q/PO)
            # weaves through pass slots.  Correctness: consumers
            # force-drain the unit queue up to their producer (the Tile
            # dep-tracker follows emission order, so a producer must be
            # emitted before its reader); the queue is constructed in
            # dependency order.
            k_unit(0, 0)()
            k_unit(1, 0)()
            q_unit(0, 0)()
            q_unit(1, 0)()

            unit_q = []
            unit_q.append((("k", 1), k_unit(0, 1)))
            unit_q.append((("k", 1), k_unit(1, 1)))
            for tk in range(2):
                unit_q.append((("v", tk), v_unit(tk)))
            unit_q.append((("k", 2), k_unit(0, 2)))
            unit_q.append((("k", 2), k_unit(1, 2)))
            for tk in range(2, 4):
                unit_q.append((("v", tk), v_unit(tk)))
            unit_q.append((("k", 3), k_unit(0, 3)))
            unit_q.append((("k", 3), k_unit(1, 3)))
            for tk in range(4, 16):
                unit_q.append((("v", tk), v_unit(tk)))

            def pop_unit():
                key, u = unit_q.pop(0)
                u()

            def ensure(key):
                # units sharing a key (both dc of a K/Q chunk) drain to
                # the LAST matching entry
                while any(k_ == key for k_, _ in unit_q):
                    pop_unit()

            def pop_y(lag):
                if len(yq) > lag:
                    kk, hh_, tt = yq.pop(0)
                    ensure(("v", tt))
                    emit_y(kk, hh_, tt)
                    return True
                return False

            passes = [(k, hc) for k in range(NQ) for hc in range(DC)]
            npass = len(passes)
            yq = []
            for pi, (k, hc) in enumerate(passes):
                last = pi == npass - 1
                lag = 8 if pi == 0 else (2 if last else LAG)
                ensure(("q", k))
                normed = pi == 0
                po_queued = pi == 0
                for tk in range(NT):
                    ensure(("k", tk // 4))
                    emit_sexp(k, hc, tk)
                    yq.append((k, hc, tk))
                    pop_y(lag)
                    if not normed and yq[0][:2] != passes[pi - 1]:
                        # previous pass fully drained by the natural pops
                        emit_norm(*passes[pi - 1])
                        normed = True
                    elif normed and not po_queued:
                        # norm done last slot; queue the output projection
                        # (hc==1 passes finalize tq k-1)
                        po_queued = True
                        if hc == 1 and k >= 1:
                            for tch in range(4):
                                unit_q.append(
                                    (("po", k - 1), po_unit((k - 1) * 4 + tch))
                                )
                    if tk == 2 and hc == 1 and k + 1 < NQ:
                        unit_q.append((("q", k + 1), q_unit(0, k + 1)))
                        unit_q.append((("q", k + 1), q_unit(1, k + 1)))
                    if len(yq) > lag + 2:
                        pop_y(lag)
                    if unit_q and (pi <= 1 or tk % 2 == 1) and (pi == 0 or tk > 2):
                        pop_unit()
                    if unit_q and len(unit_q) > 8 and tk % 2 == 0:
                        pop_unit()
                    if last and tk >= 12:
                        while unit_q:
                            pop_unit()
                        pop_y(2)
            while yq:
                kk, hh_, tt = yq.pop(0)
                ensure(("v", tt))
                emit_y(kk, hh_, tt)
            emit_norm(NQ - 1, DC - 1, on_pe=True)
            while unit_q:
                pop_unit()
            for tch in range(4):
                eng = nc.sync if tch % 2 == 0 else nc.scalar
                po_unit((NQ - 1) * 4 + tch, dma_eng=eng, on_act=True)()

            ps_w.release()
            ps_d.release()
            ps_y.release()
            ps_s.release()

    nc.compile()
    return nc


def _get_nc():
    if "nc" not in _CACHE:
        _CACHE["nc"] = _build()
    return _CACHE["nc"]


def _bf16(a):
    import ml_dtypes

    return np.ascontiguousarray(np.asarray(a, dtype=np.float32)).astype(
        ml_dtypes.bfloat16
    )


def _shard_inputs(x_q, x_kv, Wq, bq, Wkv, bkv):
    x_q = np.asarray(x_q, dtype=np.float32)
    x_kv = np.asarray(x_kv, dtype=np.float32)
    Wq = np.asarray(Wq, dtype=np.float32)
    bq = np.asarray(bq, dtype=np.float32)
    Wkv = np.asarray(Wkv, dtype=np.float32)
    bkv = np.asarray(bkv, dtype=np.float32)
    in_maps = []
    for core in range(NCORES):
        b = core // TPG
        g = core % TPG
        cols = slice(g * CL, (g + 1) * CL)
        in_maps.append(
            {
                "xq": _bf16(x_q[b]),
                "xkv": _bf16(x_kv[b]),
                "wq": _bf16(Wq[:, cols]),
                "wk": _bf16(Wkv[:, :C][:, cols]),
                "wv": _bf16(Wkv[:, C:][:, cols]),
                "wo": None,  # filled by caller (needs Wo)
                "bq": np.ascontiguousarray(bq[cols]),
                "bk": np.ascontiguousarray(bkv[:C][cols]),
            }
        )
    return in_maps


def kernel(x_q, x_kv, Wq, bq, Wkv, bkv, Wo, bo):
    from concourse.bass_utils import run_bass_kernel_spmd

    Wo = np.asarray(Wo, dtype=np.float32)
    bo = np.asarray(bo, dtype=np.float32)
    bkv = np.asarray(bkv, dtype=np.float32)

    nc = _get_nc()
    in_maps = _shard_inputs(x_q, x_kv, Wq, bq, Wkv, bkv)
    for core in range(NCORES):
        g = core % TPG
        in_maps[core]["wo"] = _bf16(Wo[g * CL : (g + 1) * CL, :])

    res = run_bass_kernel_spmd(nc, in_maps, core_ids=list(range(NCORES)))

    # host-side gather: sum tensor-parallel partials; add exact bias terms
    bias_full = bkv[C:] @ Wo + bo  # v-bias through Wo, plus output bias
    out = np.zeros((B, T, C), dtype=np.float32)
    for core in range(NCORES):
        out[core // TPG] += np.asarray(res.results[core]["out"]).astype(np.float32)
    out += bias_full[None, None, :]
    return out
